# revision 18
# baseline (speedup 1.0000x reference)
"""Trainium2 Bass kernel for a dense transformer block (LN -> causal MHA -> LN -> MLP).

Full shapes: x [2, 2048, 1024], 16 heads (dk=64), MLP hidden 4096, fp32 reference.

Sharding (8 cores): data-parallel over batch (2 groups of 4 cores), tensor-parallel
within each group: 4 heads per core for attention (Megatron column-split QKV,
row-split W_o partial + ReduceScatter over token shards), then the post-attention
residual/LN2/MLP runs token-parallel (512 tokens per core, full MLP weights
streamed from HBM).  The ReduceScatter is the only collective; final output
shards are gathered on the host.

The matmul datapath runs in bf16 (weights cast on host, activations cast
on-chip); the residual stream, LN statistics and softmax denominators stay
fp32.  PE transposes and DVE traffic run 2x faster in bf16 and the streamed
MLP weights halve to 16 MB; rel-err lands ~1e-3 against the fp32 reference.

Softmax uses no max-subtraction (scores are tiny: |s| < ~3), causal masking
zeroes exp() via a GPSIMD affine_select, and the softmax denominator comes
from an appended ones-column on V (row 64 of the AV psum accumulator).  The
reciprocal denominator is broadcast across partitions with a ones-stationary
PE matmul into PSUM (no DRAM bounce).  Diagonal key-blocks restrict scores/
exp/AV to the unmasked query range.  LN1/QKV for token column t is
interleaved with attention for query column t so the phases pipeline.
"""

import contextlib

import numpy as np
import ml_dtypes

import concourse.bass as bass
import concourse.mybir as mybir
import concourse.tile as tile
from concourse import bacc
from concourse import bass_utils
from concourse.masks import make_identity

B, S, H, NH, DK, FF = 2, 2048, 1024, 16, 64, 4096
TP, DP, NCORES = 4, 2, 8
TS = S // TP  # 512 tokens per core in the token-parallel phase
FQ = (NH // TP) * DK  # 256 q/k/v features per core (4 heads)
EPS = 1e-5
GROUPS = [[0, 1, 2, 3], [4, 5, 6, 7]]

F32 = mybir.dt.float32
F32R = mybir.dt.float32r
BF16 = mybir.dt.bfloat16
NPBF16 = ml_dtypes.bfloat16
AF = mybir.ActivationFunctionType
OP = mybir.AluOpType
AX = mybir.AxisListType


def build(nc, repeat=1, phases="ABC"):
    d = lambda name, shape: nc.dram_tensor(name, shape, F32, kind="ExternalInput").ap()
    db = lambda name, shape: nc.dram_tensor(name, shape, BF16, kind="ExternalInput").ap()
    x = db("x", [S, H])
    xs = d("xs", [TS, H])
    wq = db("wq", [H, FQ])
    wk = db("wk", [H, FQ])
    wv = db("wv", [H, FQ])
    bq = d("bq", [2, 128])
    bk = d("bk", [2, 128])
    bv = db("bv", [1, FQ])
    wo = db("wo", [FQ, H])
    wfc = db("wfc", [H, FF])
    bfc = d("bfc", [FF // 128, 128])
    wproj = db("wproj", [FF, H])
    bproj = db("bproj", [1, H])
    out = nc.dram_tensor("out", [TS, H], F32, kind="ExternalOutput").ap()

    opart = nc.dram_tensor("opart", [S, H], BF16, kind="Internal").ap()
    oshard = tuple(
        nc.dram_tensor(f"oshard_{i}", [TS // 4, H], BF16, kind="Internal").ap()
        for i in range(4)
    )

    with tile.TileContext(nc) as tc:
        for _ in range(repeat):
            _build(tc, x, xs, wq, wk, wv, bq, bk, bv, wo,
                   wfc, bfc, wproj, bproj, out, opart, oshard,
                   phases=phases)
    return nc


def _ln_stage1(nc, pools, xt, width=H):
    """Row sums + sum-of-squares for one [128, width] tile."""
    stats, scratch, lnp = pools
    s1 = stats.tile([128, 1], F32, tag="s1")
    nc.vector.reduce_sum(s1[:], xt[:], axis=AX.X)
    sq = scratch.tile([128, width], F32, tag="sq")
    s2 = stats.tile([128, 1], F32, tag="s2")
    nc.scalar.activation(sq[:], xt[:], AF.Square, accum_out=s2[:])
    return (xt, s1, s2)


def _ln_stage2(nc, pools, st, eps_sb, width=H):
    """Finish layernorm from stage-1 stats -> (x-mu)*rsqrt(var+eps).
    The affine w/b of the reference layernorm is folded into the downstream
    projection weights on the host."""
    stats, scratch, lnp = pools
    xt, s1, s2 = st
    negmu = stats.tile([128, 1], F32, tag="negmu")
    nc.scalar.mul(negmu[:], s1[:], -1.0 / width)
    mu2 = stats.tile([128, 1], F32, tag="mu2")
    nc.scalar.activation(mu2[:], s1[:], AF.Square, scale=1.0 / width)
    nmu2 = stats.tile([128, 1], F32, tag="nmu2")
    # nmu2 = EPS - mu2  (var = s2/width - mu^2; +EPS folded in)
    nc.scalar.activation(nmu2[:], mu2[:], AF.Identity, scale=-1.0, bias=eps_sb[:])
    std = stats.tile([128, 1], F32, tag="std")
    nc.scalar.activation(std[:], s2[:], AF.Sqrt, scale=1.0 / width, bias=nmu2[:])
    rinv = stats.tile([128, 1], F32, tag="rinv")
    nc.vector.reciprocal(rinv[:], std[:])
    lnt = lnp.tile([128, width], BF16, tag="ln")
    nc.vector.tensor_scalar(lnt[:], xt[:], negmu[:], rinv[:], OP.add, OP.mult)
    return lnt


def _build(tc, x, xs, wq, wk, wv, bq, bk, bv, wo,
           wfc, bfc, wproj, bproj, out, opart, oshard, phases="ABC"):
    nc = tc.nc
    with tc.tile_pool(name="consts", bufs=1) as consts:
        ident = consts.tile([128, 128], BF16, tag="ident")
        make_identity(nc, ident[:])
        eps_sb = consts.tile([128, 1], F32, tag="eps")
        nc.gpsimd.memset(eps_sb[:], EPS)
        ones64 = consts.tile([1, 64], BF16, tag="ones64")
        nc.gpsimd.memset(ones64[:], 1.0)

        # ------------- Phase AB: LN1+QKV interleaved with attention ---------
        with contextlib.ExitStack() as _stk:
            _ep = lambda name, bufs, **kw: _stk.enter_context(
                tc.tile_pool(name=name, bufs=bufs, **kw))
            qkvout = _ep("qkvout", 1)
            xin = _ep("xin", 4)
            lnp = _ep("lnb", 2)
            lnTp = _ep("lnT", 2)
            wqkv = _ep("wqkv", 1)
            stats = _ep("stats", 6)
            scratch = _ep("scratch", 2)
            cA = _ep("constsA", 1)
            aB = _ep("attnB", 1)
            epool = _ep("epool", 6)
            rpool = _ep("rpool", 2)
            x2p = _ep("x2p", 1)
            cin = _ep("cin", 2)
            ln2Tp = _ep("ln2Tp", 1)
            h1p = _ep("h1p", 1)
            wstream = _ep("wstream", 3)
            outp = _ep("outp", 2)
            cC = _ep("constsC", 1)
            ps512 = _ep("ps512", 4, space="PSUM")
            psT = _ep("psT", 2, space="PSUM")
            psAV = _ep("psAV", 2, space="PSUM")
            QT = qkvout.tile([128, 2, S], BF16, tag="QT")
            KT = qkvout.tile([128, 2, S], BF16, tag="KT")
            VA = qkvout.tile([128, 16, 4, 65], BF16, tag="VA")
            AOT = aB.tile([128, 2, S], BF16, tag="AOT")
            nc.gpsimd.memset(VA[:, :, :, 64:65], 1.0)

            # x tiles for the first column go first so the LN pipeline can
            # start before the (slow, partition-broadcast) constant loads
            xts0 = []
            for u in range(4):
                xt = xin.tile([128, H], BF16, tag="x", name=f"x0_{u}")
                nc.sync.dma_start(xt[:], x[128 * u : 128 * (u + 1), :])
                xts0.append(xt)

            bq_sb = cA.tile([128, 2], F32, tag="bq")
            nc.sync.dma_start(bq_sb[:], bq.rearrange("c p -> p c"))
            bk_sb = cA.tile([128, 2], F32, tag="bk")
            nc.sync.dma_start(bk_sb[:], bk.rearrange("c p -> p c"))
            bv_sb = cA.tile([128, FQ], BF16, tag="bv")
            nc.sync.dma_start(bv_sb[:], bv[0, :].partition_broadcast(128))

            wq_sb = wqkv.tile([128, 8, FQ], BF16, tag="wq")
            nc.sync.dma_start(wq_sb[:], wq.rearrange("(h p) f -> p h f", p=128))
            wk_sb = wqkv.tile([128, 8, FQ], BF16, tag="wk")
            nc.sync.dma_start(wk_sb[:], wk.rearrange("(h p) f -> p h f", p=128))
            wv_sb = wqkv.tile([128, 8, FQ], BF16, tag="wv")
            nc.sync.dma_start(wv_sb[:], wv.rearrange("(h p) f -> p h f", p=128))
            wo_sb = aB.tile([128, 2, H], BF16, tag="wo")
            nc.sync.dma_start(wo_sb[:], wo.rearrange("(c p) f -> p c f", p=128))
            if phases in ("ABC", "ABX"):
                bproj_b = cC.tile([128, H], BF16, tag="bproj_b")
                nc.sync.dma_start(bproj_b[:],
                                  bproj[0, :].partition_broadcast(128))
                bfc_sb = cC.tile([128, FF // 128], F32, tag="bfc")
                nc.sync.dma_start(bfc_sb[:], bfc.rearrange("c p -> p c"))

            lnpools = (stats, scratch, lnp)

            def _emit_oproj(qc):
                # partial O-projection for token column qc (deferred one
                # column so the softmax epilogue hides behind the next
                # column's LN/QKV work on the PE)
                for u in range(4):
                    t = 4 * qc + u
                    for oc in range(2):
                        po = ps512.tile([128, TS], F32, tag="s",
                                        name=f"po{qc}_{u}_{oc}")
                        for c in range(2):
                            nc.tensor.matmul(
                                po[:],
                                AOT[:, c, 128 * t : 128 * (t + 1)],
                                wo_sb[:, c, TS * oc : TS * (oc + 1)],
                                start=(c == 0),
                                stop=(c == 1),
                            )
                        ost = epool.tile([128, TS], BF16, tag="ost",
                                         name=f"ost{qc}_{u}_{oc}")
                        nc.vector.tensor_copy(ost[:], po[:])
                        nc.sync.dma_start(
                            opart[128 * t : 128 * (t + 1),
                                  TS * oc : TS * (oc + 1)],
                            ost[:],
                        )
                # per-column ReduceScatter: columns 0-2 overlap the
                # remaining attention columns; only column 3's is exposed
                # ("ABX" = timing-only probe: full compute, collective
                # skipped, phase C reads stale oshard garbage)
                if phases == "ABC":
                    nc.gpsimd.collective_compute(
                        "ReduceScatter", OP.add, replica_groups=GROUPS,
                        ins=[opart[TS * qc : TS * (qc + 1), :]],
                        outs=[oshard[qc][:]],
                    )

            for tcn in range(4):
                if tcn == 3 and phases in ("ABC", "ABX"):
                    # prefetch the first FC weight chunks; they load during
                    # attention column 3 so FC starts unstalled
                    wfc_pre = []
                    for g in range(2):
                        wt = wstream.tile([128, 8, TS], BF16, tag="wst",
                                          name=f"wfc0_{g}")
                        nc.sync.dma_start(
                            wt[:],
                            wfc.rearrange("(h p) f -> p h f", p=128)[
                                :, :, TS * g : TS * (g + 1)
                            ],
                        )
                        wfc_pre.append(wt)

                # --- LN1 + batched transpose + QKV for token column tcn ---
                lnT = lnTp.tile([128, 8, TS], BF16, tag="lnT", name=f"lnT{tcn}")

                def _finish_ln1(u, st, lnT=lnT):
                    lnt = _ln_stage2(nc, lnpools, st, eps_sb)
                    for f in range(8):
                        pt = psT.tile([128, 128], BF16, tag="pt")
                        nc.tensor.transpose(
                            pt[:], lnt[:, 128 * f : 128 * (f + 1)], ident[:]
                        )
                        dst_ap = lnT[:, f, 128 * u : 128 * (u + 1)]
                        nc.vector.tensor_copy(dst_ap, pt[:])

                pend = []
                for u in range(4):
                    t = 4 * tcn + u
                    if tcn == 0:
                        xt = xts0[u]
                    else:
                        xt = xin.tile([128, H], BF16, tag="x")
                        nc.sync.dma_start(xt[:], x[128 * t : 128 * (t + 1), :])
                    pend.append((u, _ln_stage1(nc, lnpools, xt)))
                    if len(pend) > 1:
                        _finish_ln1(*pend.pop(0))
                _finish_ln1(*pend.pop(0))
                tsl = slice(TS * tcn, TS * (tcn + 1))
                for (wt, dst, bias) in ((wq_sb, QT, bq_sb), (wk_sb, KT, bk_sb)):
                    for c in range(2):
                        pq = ps512.tile([128, TS], F32, tag="s")
                        for ht in range(8):
                            nc.tensor.matmul(
                                pq[:],
                                wt[:, ht, 128 * c : 128 * (c + 1)],
                                lnT[:, ht, :],
                                start=(ht == 0),
                                stop=(ht == 7),
                            )
                        nc.scalar.activation(
                            dst[:, c, tsl], pq[:], AF.Identity,
                            bias=bias[:, c : c + 1],
                        )
                for u in range(4):
                    t = 4 * tcn + u
                    pv = ps512.tile([128, FQ], F32, tag="s", name=f"pv{t}")
                    for ht in range(8):
                        nc.tensor.matmul(
                            pv[:],
                            lnT[:, ht, 128 * u : 128 * (u + 1)],
                            wv_sb[:, ht, :],
                            start=(ht == 0),
                            stop=(ht == 7),
                        )
                    nc.vector.tensor_add(
                        VA[:, t, :, 0:64],
                        pv[:].rearrange("p (h f) -> p h f", h=4),
                        bv_sb[:].rearrange("p (h f) -> p h f", h=4),
                    )

                if phases == "A":
                    continue
                if tcn > 0:
                    _emit_oproj(tcn - 1)
                # --- attention for query column qc = tcn ---
                qc = tcn
                nkb = 4 * qc + 4
                for hp in range(2):
                    pvs = [
                        psAV.tile([65, TS], F32, tag="pav",
                                  name=f"pav{qc}_{hp}_{i}")
                        for i in range(2)
                    ]
                    def _emit_av(kb, q0, es):
                        for hh in range(2):
                            h = 2 * hp + hh
                            nc.tensor.matmul(
                                pvs[hh][:, q0:],
                                VA[:, kb, h, :],
                                es[hh][:, q0:],
                                start=(kb == 0),
                                stop=(kb == nkb - 1),
                                skip_group_check=True,
                            )

                    prevs = []
                    for kb in range(nkb):
                        # diagonal key-blocks: queries below 128*dd are fully
                        # masked; restrict scores/exp/mask/AV to [q0:]
                        dd = kb - 4 * qc
                        q0 = 128 * dd if dd > 0 else 0
                        es = []
                        for hh in range(2):
                            base = 64 * hh
                            sp = ps512.tile([128, TS], F32, tag="s")
                            nc.tensor.matmul(
                                sp[:, q0:],
                                KT[base : base + 64, hp,
                                   128 * kb : 128 * (kb + 1)],
                                QT[base : base + 64, hp,
                                   TS * qc + q0 : TS * (qc + 1)],
                                start=True,
                                stop=True,
                                tile_position=(base, 0),
                            )
                            e = epool.tile([128, TS], BF16, tag="e")
                            nc.scalar.activation(e[:, q0:], sp[:, q0:],
                                                 AF.Exp, scale=0.125)
                            if dd >= 0:
                                nc.gpsimd.affine_select(
                                    e[:, q0:], e[:, q0:],
                                    pattern=[[1, TS - q0]],
                                    compare_op=OP.is_ge,
                                    fill=0.0,
                                    base=0,
                                    channel_multiplier=-1,
                                )
                            es.append(e)
                        prevs.append((kb, q0, es))
                        if len(prevs) > 2:
                            _emit_av(*prevs.pop(0))
                    while prevs:
                        _emit_av(*prevs.pop(0))
                    qsl = slice(TS * qc, TS * (qc + 1))
                    for hh in range(2):
                        base = 64 * hh
                        # copy psum out immediately so the AV accumulator bank
                        # frees; the reciprocal row broadcasts back across 64
                        # partitions via a ones-stationary PE matmul into PSUM
                        nv = rpool.tile([65, TS], F32, tag="nv")
                        nc.vector.tensor_copy(nv[:], pvs[hh][:])
                        rd = rpool.tile([1, TS], BF16, tag="rd")
                        with nc.allow_low_precision(
                                reason="softmax denom reciprocal in bf16"):
                            nc.vector.reciprocal(rd[:], nv[64:65, :])
                        pb = psAV.tile([64, TS], F32, tag="pav",
                                       name=f"pb{qc}_{hp}_{hh}")
                        nc.tensor.matmul(pb[:], ones64[:], rd[:],
                                         start=True, stop=True)
                        nc.vector.tensor_mul(
                            AOT[base : base + 64, hp, qsl],
                            nv[0:64, :],
                            pb[:],
                        )


            if phases != "A":
                _emit_oproj(3)

            if phases not in ("ABC", "ABX"):
                return

            # ---- Phase C (merged): residual + LN2 + MLP in token halves.
            # Slabs 0-2's ReduceScatters completed during the attention
            # columns, so the first half's FC/proj runs under the tail
            # ReduceScatter; only slab 3 waits for it. wfc/wproj stream
            # twice (once per half) to keep SBUF small.
            x2 = x2p.tile([128, 4, H], F32, tag="x2")
            ln2T = ln2Tp.tile([128, 8, TS], BF16, tag="ln2T")
            h1T = h1p.tile([128, 32, TS], BF16, tag="h1T")

            def _finish_ln2(u, st):
                lnt = _ln_stage2(nc, lnpools, st, eps_sb)
                for f in range(8):
                    pt = psT.tile([128, 128], BF16, tag="pt",
                                  name=f"pt2_{u}_{f}")
                    nc.tensor.transpose(
                        pt[:], lnt[:, 128 * f : 128 * (f + 1)], ident[:]
                    )
                    nc.vector.tensor_copy(
                        ln2T[:, f, 128 * u : 128 * (u + 1)], pt[:])
                # x2's LN reads are done: fold the proj bias into the
                # residual in place so the epilogue is a single add
                nc.vector.tensor_add(x2[:, u, :], x2[:, u, :], bproj_b[:])

            def _emit_ln2(u):
                ot = cin.tile([128, H], BF16, tag="ot")
                nc.sync.dma_start(ot[:], oshard[u][:])
                xst = cin.tile([128, H], F32, tag="xst")
                nc.sync.dma_start(xst[:], xs[128 * u : 128 * (u + 1), :])
                nc.vector.tensor_add(x2[:, u, :], ot[:], xst[:])
                return (u, _ln_stage1(nc, lnpools, x2[:, u, :]))

            def _emit_fc(half, pf_alloc):
                q0 = 256 * half
                for g in range(8):
                    if half == 0 and g < 2:
                        wt = wfc_pre[g]
                    else:
                        wt = wstream.tile([128, 8, TS], BF16, tag="wst",
                                          name=f"wfc{half}_{g}")
                        nc.sync.dma_start(
                            wt[:],
                            wfc.rearrange("(h p) f -> p h f", p=128)[
                                :, :, TS * g : TS * (g + 1)
                            ],
                        )
                    for c4 in range(4):
                        fc = 4 * g + c4
                        pf = pf_alloc(f"pf{half}_{fc}")
                        for ht in range(8):
                            nc.tensor.matmul(
                                pf[:],
                                wt[:, ht, 128 * c4 : 128 * (c4 + 1)],
                                ln2T[:, ht, q0 : q0 + 256],
                                start=(ht == 0),
                                stop=(ht == 7),
                            )
                        nc.scalar.activation(
                            h1T[:, fc, q0 : q0 + 256], pf[:], AF.Gelu,
                            bias=bfc_sb[:, fc : fc + 1],
                        )

            def _emit_proj(half, accs):
                us = (0, 1) if half == 0 else (2, 3)
                for g in range(8):
                    wt = wstream.tile([128, 4, H], BF16, tag="wst",
                                      name=f"wpj{half}_{g}")
                    nc.sync.dma_start(
                        wt[:],
                        wproj.rearrange("(c p) f -> p c f", p=128)[
                            :, 4 * g : 4 * (g + 1), :
                        ],
                    )
                    for iu, u in enumerate(us):
                        for oc in range(2):
                            for f4 in range(4):
                                fc = 4 * g + f4
                                nc.tensor.matmul(
                                    accs[2 * iu + oc][:],
                                    h1T[:, fc, 128 * u : 128 * (u + 1)],
                                    wt[:, f4, TS * oc : TS * (oc + 1)],
                                    start=(g == 0 and f4 == 0),
                                    stop=(g == 7 and f4 == 3),
                                    skip_group_check=True,
                                )
                            if g == 7:
                                # drain each finished accumulator while the
                                # PE works on the remaining pairs
                                osl = slice(TS * oc, TS * (oc + 1))
                                ro = outp.tile([128, TS], F32, tag="ro",
                                               name=f"ro{u}_{oc}")
                                nc.vector.tensor_add(
                                    ro[:], accs[2 * iu + oc][:],
                                    x2[:, u, osl])
                                nc.sync.dma_start(
                                    out[128 * u : 128 * (u + 1), osl], ro[:])

            pend2 = [_emit_ln2(0)]
            pend2.append(_emit_ln2(1))
            _finish_ln2(*pend2.pop(0))
            pend2.append(_emit_ln2(2))
            _finish_ln2(*pend2.pop(0))
            _emit_fc(0, lambda nm: ps512.tile([128, 256], F32, tag="s",
                                              name=nm))
            accs0 = [ps512.tile([128, TS], F32, tag="s", name=f"acc0_{i}")
                     for i in range(4)]
            _emit_proj(0, accs0)
            pend2.append(_emit_ln2(3))
            _finish_ln2(*pend2.pop(0))
            _finish_ln2(*pend2.pop(0))
            _emit_fc(1, lambda nm: psT.tile([128, 256], F32, tag="pt",
                                            name=nm))
            accs1 = [psT.tile([128, TS], F32, tag="pt", name="acc1_0"),
                     psT.tile([128, TS], F32, tag="pt", name="acc1_1"),
                     psAV.tile([128, TS], F32, tag="pav", name="acc1_2"),
                     psAV.tile([128, TS], F32, tag="pav", name="acc1_3")]
            _emit_proj(1, accs1)


_CACHE = {}


def _get_compiled():
    if "nc" not in _CACHE:
        nc = bacc.Bacc("TRN2", target_bir_lowering=False, debug=False,
                       num_devices=NCORES)
        build(nc)
        nc.compile()
        _CACHE["nc"] = nc
    return _CACHE["nc"]


def make_in_maps(x, ln1_w, ln1_b, W_qkv, b_qkv, W_o, b_o, ln2_w, ln2_b, W_fc,
                 b_fc, W_proj, b_proj):
    x = np.ascontiguousarray(np.asarray(x, np.float32))
    ln1_w = np.asarray(ln1_w, np.float32)
    ln1_b = np.asarray(ln1_b, np.float32)
    ln2_w = np.asarray(ln2_w, np.float32)
    ln2_b = np.asarray(ln2_b, np.float32)
    W_qkv_raw = np.asarray(W_qkv, np.float32)
    W_fc_raw = np.asarray(W_fc, np.float32)
    # fold the layernorm affines into the downstream projections:
    # (c*w + b) @ W == c @ (diag(w) W) + b @ W
    W_qkv = ln1_w[:, None] * W_qkv_raw
    b_qkv = np.asarray(b_qkv, np.float32) + ln1_b @ W_qkv_raw
    bf = lambda a: np.ascontiguousarray(np.asarray(a).astype(NPBF16))
    shared = {
        "wfc": bf(ln2_w[:, None] * W_fc_raw),
        "bfc": np.ascontiguousarray(
            (np.asarray(b_fc, np.float32) + ln2_b @ W_fc_raw
             ).reshape(FF // 128, 128)),
        "wproj": bf(W_proj),
        "bproj": bf(np.asarray(b_proj, np.float32).reshape(1, H)),
    }
    in_maps = []
    for c in range(NCORES):
        b, r = c // TP, c % TP
        fsl = slice(FQ * r, FQ * (r + 1))
        m = dict(shared)
        m["x"] = bf(x[b])
        m["xs"] = np.ascontiguousarray(np.concatenate(
            [x[b][TS * u + 128 * r : TS * u + 128 * (r + 1)]
             for u in range(4)], axis=0) + np.asarray(b_o, np.float32)[None, :])
        m["wq"] = bf(W_qkv[:, fsl])
        m["wk"] = bf(W_qkv[:, H:][:, fsl])
        m["wv"] = bf(W_qkv[:, 2 * H :][:, fsl])
        m["bq"] = np.ascontiguousarray(b_qkv[fsl].reshape(2, 128))
        m["bk"] = np.ascontiguousarray(b_qkv[H:][fsl].reshape(2, 128))
        m["bv"] = bf(b_qkv[2 * H :][fsl].reshape(1, FQ))
        m["wo"] = bf(np.asarray(W_o, np.float32)[fsl, :])
        in_maps.append(m)
    return in_maps


def kernel(**inputs):
    nc = _get_compiled()
    in_maps = make_in_maps(**inputs)
    res = bass_utils.run_bass_kernel_spmd(
        nc, in_maps, core_ids=list(range(NCORES)), trace=False
    )
    out = np.empty((B, S, H), np.float32)
    for c in range(NCORES):
        b, r = c // TP, c % TP
        o = res.results[c]["out"]
        for u in range(4):
            out[b, TS * u + 128 * r : TS * u + 128 * (r + 1), :] = \
                o[128 * u : 128 * (u + 1)]
    return out


# revision 20
# speedup vs baseline: 1.1348x; 1.1348x over previous
"""Trainium2 Bass kernel for a dense transformer block (LN -> causal MHA -> LN -> MLP).

Full shapes: x [2, 2048, 1024], 16 heads (dk=64), MLP hidden 4096, fp32 reference.

Sharding (8 cores): data-parallel over batch (2 groups of 4 cores), tensor-parallel
within each group: 4 heads per core for attention (Megatron column-split QKV,
row-split W_o partial + ReduceScatter over token shards), then the post-attention
residual/LN2/MLP runs token-parallel (512 tokens per core, full MLP weights
streamed from HBM).  The ReduceScatter is the only collective; final output
shards are gathered on the host.

The matmul datapath runs in bf16 (weights cast on host, activations cast
on-chip); the residual stream, LN statistics and softmax denominators stay
fp32.  PE transposes and DVE traffic run 2x faster in bf16 and the streamed
MLP weights halve to 16 MB; rel-err lands ~1e-3 against the fp32 reference.

Softmax uses no max-subtraction (scores are tiny: |s| < ~3), causal masking
zeroes exp() via a GPSIMD affine_select, and the softmax denominator comes
from an appended ones-column on V (row 64 of the AV psum accumulator).  The
reciprocal denominator is broadcast across partitions with a ones-stationary
PE matmul into PSUM (no DRAM bounce).  Diagonal key-blocks restrict scores/
exp/AV to the unmasked query range.  LN1/QKV for token column t is
interleaved with attention for query column t so the phases pipeline.
"""

import contextlib

import numpy as np
import ml_dtypes

import concourse.bass as bass
import concourse.mybir as mybir
import concourse.tile as tile
from concourse import bacc
from concourse import bass_utils
from concourse.masks import make_identity

B, S, H, NH, DK, FF = 2, 2048, 1024, 16, 64, 4096
TP, DP, NCORES = 4, 2, 8
TS = S // TP  # 512 tokens per core in the token-parallel phase
FQ = (NH // TP) * DK  # 256 q/k/v features per core (4 heads)
EPS = 1e-5
GROUPS = [[0, 1, 2, 3], [4, 5, 6, 7]]

F32 = mybir.dt.float32
F32R = mybir.dt.float32r
BF16 = mybir.dt.bfloat16
NPBF16 = ml_dtypes.bfloat16
AF = mybir.ActivationFunctionType
OP = mybir.AluOpType
AX = mybir.AxisListType


def build(nc, repeat=1, phases="ABC"):
    d = lambda name, shape: nc.dram_tensor(name, shape, F32, kind="ExternalInput").ap()
    db = lambda name, shape: nc.dram_tensor(name, shape, BF16, kind="ExternalInput").ap()
    x = db("x", [S, H])
    xs = d("xs", [TS, H])
    wq = db("wq", [H, FQ])
    wk = db("wk", [H, FQ])
    wv = db("wv", [H, FQ])
    bq = d("bq", [2, 128])
    bk = d("bk", [2, 128])
    bv = db("bv", [1, FQ])
    wo = db("wo", [FQ, H])
    wfc = db("wfc", [H, FF])
    bfc = d("bfc", [FF // 128, 128])
    wproj = db("wproj", [FF, H])
    bproj = db("bproj", [1, H])
    out = nc.dram_tensor("out", [TS, H], F32, kind="ExternalOutput").ap()

    # O-projection partials, laid out in ReduceScatter scatter order:
    # opartA row = 384*slab + 128*col + p for columns 0-2 (so one contiguous
    # RS hands core r its three slabs), opartB holds column 3 for the tail RS
    opartA = nc.dram_tensor("opart_a", [3 * TS, H], BF16, kind="Internal").ap()
    opartB = nc.dram_tensor("opart_b", [TS, H], BF16, kind="Internal").ap()
    opart = (opartA, opartB)
    # oshardA: one strided-scatter ReduceScatter covers columns 0-2 (each
    # core receives its three 128-row slabs, column-major); oshardB covers
    # column 3 alone so the tail collective moves minimal data.
    oshardA = nc.dram_tensor("oshard_a", [3 * (TS // 4), H], BF16,
                             kind="Internal").ap()
    oshardB = nc.dram_tensor("oshard_b", [TS // 4, H], BF16,
                             kind="Internal").ap()
    oshard = (oshardA, oshardB)

    with tile.TileContext(nc) as tc:
        for _ in range(repeat):
            _build(tc, x, xs, wq, wk, wv, bq, bk, bv, wo,
                   wfc, bfc, wproj, bproj, out, opart, oshard,
                   phases=phases)
    return nc


def _ln_stage1(nc, pools, xt, width=H):
    """Row sums + sum-of-squares for one [128, width] tile."""
    stats, scratch, lnp = pools
    s1 = stats.tile([128, 1], F32, tag="s1")
    nc.vector.reduce_sum(s1[:], xt[:], axis=AX.X)
    sq = scratch.tile([128, width], F32, tag="sq")
    s2 = stats.tile([128, 1], F32, tag="s2")
    nc.scalar.activation(sq[:], xt[:], AF.Square, accum_out=s2[:])
    return (xt, s1, s2)


def _ln_stage2(nc, pools, st, eps_sb, width=H):
    """Finish layernorm from stage-1 stats -> (x-mu)*rsqrt(var+eps).
    The affine w/b of the reference layernorm is folded into the downstream
    projection weights on the host."""
    stats, scratch, lnp = pools
    xt, s1, s2 = st
    negmu = stats.tile([128, 1], F32, tag="negmu")
    nc.scalar.mul(negmu[:], s1[:], -1.0 / width)
    mu2 = stats.tile([128, 1], F32, tag="mu2")
    nc.scalar.activation(mu2[:], s1[:], AF.Square, scale=1.0 / width)
    nmu2 = stats.tile([128, 1], F32, tag="nmu2")
    # nmu2 = EPS - mu2  (var = s2/width - mu^2; +EPS folded in)
    nc.scalar.activation(nmu2[:], mu2[:], AF.Identity, scale=-1.0, bias=eps_sb[:])
    std = stats.tile([128, 1], F32, tag="std")
    nc.scalar.activation(std[:], s2[:], AF.Sqrt, scale=1.0 / width, bias=nmu2[:])
    rinv = stats.tile([128, 1], F32, tag="rinv")
    nc.vector.reciprocal(rinv[:], std[:])
    lnt = lnp.tile([128, width], BF16, tag="ln")
    nc.vector.tensor_scalar(lnt[:], xt[:], negmu[:], rinv[:], OP.add, OP.mult)
    return lnt


def _build(tc, x, xs, wq, wk, wv, bq, bk, bv, wo,
           wfc, bfc, wproj, bproj, out, opart, oshard, phases="ABC"):
    nc = tc.nc
    with tc.tile_pool(name="consts", bufs=1) as consts:
        ident = consts.tile([128, 128], BF16, tag="ident")
        make_identity(nc, ident[:])
        eps_sb = consts.tile([128, 1], F32, tag="eps")
        nc.gpsimd.memset(eps_sb[:], EPS)
        ones64 = consts.tile([1, 64], BF16, tag="ones64")
        nc.gpsimd.memset(ones64[:], 1.0)

        # ------------- Phase AB: LN1+QKV interleaved with attention ---------
        with contextlib.ExitStack() as _stk:
            _ep = lambda name, bufs, **kw: _stk.enter_context(
                tc.tile_pool(name=name, bufs=bufs, **kw))
            qkvout = _ep("qkvout", 1)
            xin = _ep("xin", 4)
            lnp = _ep("lnb", 2)
            lnTp = _ep("lnT", 2)
            wqkv = _ep("wqkv", 1)
            stats = _ep("stats", 6)
            scratch = _ep("scratch", 2)
            cA = _ep("constsA", 1)
            aB = _ep("attnB", 1)
            epool = _ep("epool", 6)
            rpool = _ep("rpool", 2)
            x2p = _ep("x2p", 1)
            cin = _ep("cin", 2)
            ln2Tp = _ep("ln2Tp", 1)
            h1p = _ep("h1p", 1)
            wstream = _ep("wstream", 3)
            outp = _ep("outp", 2)
            cC = _ep("constsC", 1)
            ps512 = _ep("ps512", 4, space="PSUM")
            psT = _ep("psT", 2, space="PSUM")
            psAV = _ep("psAV", 2, space="PSUM")
            QT = qkvout.tile([128, 2, S], BF16, tag="QT")
            KT = qkvout.tile([128, 2, S], BF16, tag="KT")
            VA = qkvout.tile([128, 16, 4, 65], BF16, tag="VA")
            AOT = aB.tile([128, 2, S], BF16, tag="AOT")
            nc.gpsimd.memset(VA[:, :, :, 64:65], 1.0)

            # x tiles for the first column go first so the LN pipeline can
            # start before the (slow, partition-broadcast) constant loads
            xts0 = []
            for u in range(4):
                xt = xin.tile([128, H], BF16, tag="x", name=f"x0_{u}")
                nc.sync.dma_start(xt[:], x[128 * u : 128 * (u + 1), :])
                xts0.append(xt)

            bq_sb = cA.tile([128, 2], F32, tag="bq")
            nc.sync.dma_start(bq_sb[:], bq.rearrange("c p -> p c"))
            bk_sb = cA.tile([128, 2], F32, tag="bk")
            nc.sync.dma_start(bk_sb[:], bk.rearrange("c p -> p c"))
            bv_sb = cA.tile([128, FQ], BF16, tag="bv")
            nc.sync.dma_start(bv_sb[:], bv[0, :].partition_broadcast(128))

            wq_sb = wqkv.tile([128, 8, FQ], BF16, tag="wq")
            nc.sync.dma_start(wq_sb[:], wq.rearrange("(h p) f -> p h f", p=128))
            wk_sb = wqkv.tile([128, 8, FQ], BF16, tag="wk")
            nc.sync.dma_start(wk_sb[:], wk.rearrange("(h p) f -> p h f", p=128))
            wv_sb = wqkv.tile([128, 8, FQ], BF16, tag="wv")
            nc.sync.dma_start(wv_sb[:], wv.rearrange("(h p) f -> p h f", p=128))
            wo_sb = aB.tile([128, 2, H], BF16, tag="wo")
            nc.sync.dma_start(wo_sb[:], wo.rearrange("(c p) f -> p c f", p=128))
            if phases in ("ABC", "ABX"):
                bproj_b = cC.tile([128, H], BF16, tag="bproj_b")
                nc.sync.dma_start(bproj_b[:],
                                  bproj[0, :].partition_broadcast(128))
                bfc_sb = cC.tile([128, FF // 128], F32, tag="bfc")
                nc.sync.dma_start(bfc_sb[:], bfc.rearrange("c p -> p c"))

            lnpools = (stats, scratch, lnp)

            def _emit_oproj(qc):
                # partial O-projection for token column qc (deferred one
                # column so the softmax epilogue hides behind the next
                # column's LN/QKV work on the PE)
                for u in range(4):
                    t = 4 * qc + u
                    for oc in range(2):
                        po = ps512.tile([128, TS], F32, tag="s",
                                        name=f"po{qc}_{u}_{oc}")
                        for c in range(2):
                            nc.tensor.matmul(
                                po[:],
                                AOT[:, c, 128 * t : 128 * (t + 1)],
                                wo_sb[:, c, TS * oc : TS * (oc + 1)],
                                start=(c == 0),
                                stop=(c == 1),
                            )
                        ost = epool.tile([128, TS], BF16, tag="ost",
                                         name=f"ost{qc}_{u}_{oc}")
                        nc.vector.tensor_copy(ost[:], po[:])
                        if qc < 3:
                            dst = opart[0][384 * u + 128 * qc :
                                           384 * u + 128 * (qc + 1),
                                           TS * oc : TS * (oc + 1)]
                        else:
                            dst = opart[1][128 * u : 128 * (u + 1),
                                           TS * oc : TS * (oc + 1)]
                        nc.sync.dma_start(dst, ost[:])
                # two ReduceScatters total: columns 0-2 batched into one
                # strided-scatter RS (fires after oproj(2), fully overlapped
                # by attention column 3); column 3 alone in the tail RS.
                # ("ABX" = timing-only probe: full compute, collectives
                # skipped, phase C reads stale oshard garbage)
                if phases == "ABC" and qc == 2:
                    nc.gpsimd.collective_compute(
                        "ReduceScatter", OP.add, replica_groups=GROUPS,
                        ins=[opart[0][:]],
                        outs=[oshard[0][:]],
                    )
                if phases == "ABC" and qc == 3:
                    nc.gpsimd.collective_compute(
                        "ReduceScatter", OP.add, replica_groups=GROUPS,
                        ins=[opart[1][:]],
                        outs=[oshard[1][:]],
                    )

            for tcn in range(4):
                if tcn == 3 and phases in ("ABC", "ABX"):
                    # prefetch the first FC weight chunks; they load during
                    # attention column 3 so FC starts unstalled
                    wfc_pre = []
                    for g in range(2):
                        wt = wstream.tile([128, 8, TS], BF16, tag="wst",
                                          name=f"wfc0_{g}")
                        nc.sync.dma_start(
                            wt[:],
                            wfc.rearrange("(h p) f -> p h f", p=128)[
                                :, :, TS * g : TS * (g + 1)
                            ],
                        )
                        wfc_pre.append(wt)

                # --- LN1 + batched transpose + QKV for token column tcn ---
                lnT = lnTp.tile([128, 8, TS], BF16, tag="lnT", name=f"lnT{tcn}")

                def _finish_ln1(u, st, lnT=lnT):
                    lnt = _ln_stage2(nc, lnpools, st, eps_sb)
                    for f in range(8):
                        pt = psT.tile([128, 128], BF16, tag="pt")
                        nc.tensor.transpose(
                            pt[:], lnt[:, 128 * f : 128 * (f + 1)], ident[:]
                        )
                        dst_ap = lnT[:, f, 128 * u : 128 * (u + 1)]
                        nc.vector.tensor_copy(dst_ap, pt[:])

                pend = []
                for u in range(4):
                    t = 4 * tcn + u
                    if tcn == 0:
                        xt = xts0[u]
                    else:
                        xt = xin.tile([128, H], BF16, tag="x")
                        nc.sync.dma_start(xt[:], x[128 * t : 128 * (t + 1), :])
                    pend.append((u, _ln_stage1(nc, lnpools, xt)))
                    if len(pend) > 1:
                        _finish_ln1(*pend.pop(0))
                _finish_ln1(*pend.pop(0))
                tsl = slice(TS * tcn, TS * (tcn + 1))
                for (wt, dst, bias) in ((wq_sb, QT, bq_sb), (wk_sb, KT, bk_sb)):
                    for c in range(2):
                        pq = ps512.tile([128, TS], F32, tag="s")
                        for ht in range(8):
                            nc.tensor.matmul(
                                pq[:],
                                wt[:, ht, 128 * c : 128 * (c + 1)],
                                lnT[:, ht, :],
                                start=(ht == 0),
                                stop=(ht == 7),
                            )
                        nc.scalar.activation(
                            dst[:, c, tsl], pq[:], AF.Identity,
                            bias=bias[:, c : c + 1],
                        )
                for u in range(4):
                    t = 4 * tcn + u
                    pv = ps512.tile([128, FQ], F32, tag="s", name=f"pv{t}")
                    for ht in range(8):
                        nc.tensor.matmul(
                            pv[:],
                            lnT[:, ht, 128 * u : 128 * (u + 1)],
                            wv_sb[:, ht, :],
                            start=(ht == 0),
                            stop=(ht == 7),
                        )
                    nc.vector.tensor_add(
                        VA[:, t, :, 0:64],
                        pv[:].rearrange("p (h f) -> p h f", h=4),
                        bv_sb[:].rearrange("p (h f) -> p h f", h=4),
                    )

                if phases == "A":
                    continue
                if tcn > 0:
                    _emit_oproj(tcn - 1)
                # --- attention for query column qc = tcn ---
                qc = tcn
                nkb = 4 * qc + 4
                for hp in range(2):
                    pvs = [
                        psAV.tile([65, TS], F32, tag="pav",
                                  name=f"pav{qc}_{hp}_{i}")
                        for i in range(2)
                    ]
                    def _emit_av(kb, q0, es):
                        for hh in range(2):
                            h = 2 * hp + hh
                            nc.tensor.matmul(
                                pvs[hh][:, q0:],
                                VA[:, kb, h, :],
                                es[hh][:, q0:],
                                start=(kb == 0),
                                stop=(kb == nkb - 1),
                                skip_group_check=True,
                            )

                    prevs = []
                    for kb in range(nkb):
                        # diagonal key-blocks: queries below 128*dd are fully
                        # masked; restrict scores/exp/mask/AV to [q0:]
                        dd = kb - 4 * qc
                        q0 = 128 * dd if dd > 0 else 0
                        es = []
                        for hh in range(2):
                            base = 64 * hh
                            sp = ps512.tile([128, TS], F32, tag="s")
                            nc.tensor.matmul(
                                sp[:, q0:],
                                KT[base : base + 64, hp,
                                   128 * kb : 128 * (kb + 1)],
                                QT[base : base + 64, hp,
                                   TS * qc + q0 : TS * (qc + 1)],
                                start=True,
                                stop=True,
                                tile_position=(base, 0),
                            )
                            e = epool.tile([128, TS], BF16, tag="e")
                            nc.scalar.activation(e[:, q0:], sp[:, q0:],
                                                 AF.Exp, scale=0.125)
                            if dd >= 0:
                                nc.gpsimd.affine_select(
                                    e[:, q0:], e[:, q0:],
                                    pattern=[[1, TS - q0]],
                                    compare_op=OP.is_ge,
                                    fill=0.0,
                                    base=0,
                                    channel_multiplier=-1,
                                )
                            es.append(e)
                        prevs.append((kb, q0, es))
                        if len(prevs) > 2:
                            _emit_av(*prevs.pop(0))
                    while prevs:
                        _emit_av(*prevs.pop(0))
                    qsl = slice(TS * qc, TS * (qc + 1))
                    for hh in range(2):
                        base = 64 * hh
                        # copy psum out immediately so the AV accumulator bank
                        # frees; the reciprocal row broadcasts back across 64
                        # partitions via a ones-stationary PE matmul into PSUM
                        nv = rpool.tile([65, TS], F32, tag="nv")
                        nc.vector.tensor_copy(nv[:], pvs[hh][:])
                        rd = rpool.tile([1, TS], BF16, tag="rd")
                        with nc.allow_low_precision(
                                reason="softmax denom reciprocal in bf16"):
                            nc.vector.reciprocal(rd[:], nv[64:65, :])
                        pb = psAV.tile([64, TS], F32, tag="pav",
                                       name=f"pb{qc}_{hp}_{hh}")
                        nc.tensor.matmul(pb[:], ones64[:], rd[:],
                                         start=True, stop=True)
                        nc.vector.tensor_mul(
                            AOT[base : base + 64, hp, qsl],
                            nv[0:64, :],
                            pb[:],
                        )


            if phases != "A":
                _emit_oproj(3)

            if phases not in ("ABC", "ABX"):
                return

            # ---- Phase C (merged): residual + LN2 + MLP in token halves.
            # Slabs 0-2's ReduceScatters completed during the attention
            # columns, so the first half's FC/proj runs under the tail
            # ReduceScatter; only slab 3 waits for it. wfc/wproj stream
            # twice (once per half) to keep SBUF small.
            x2 = x2p.tile([128, 4, H], F32, tag="x2")
            ln2T = ln2Tp.tile([128, 8, TS], BF16, tag="ln2T")
            h1T = h1p.tile([128, 32, TS], BF16, tag="h1T")

            def _finish_ln2(u, st):
                lnt = _ln_stage2(nc, lnpools, st, eps_sb)
                for f in range(8):
                    pt = psT.tile([128, 128], BF16, tag="pt",
                                  name=f"pt2_{u}_{f}")
                    nc.tensor.transpose(
                        pt[:], lnt[:, 128 * f : 128 * (f + 1)], ident[:]
                    )
                    nc.vector.tensor_copy(
                        ln2T[:, f, 128 * u : 128 * (u + 1)], pt[:])
                # x2's LN reads are done: fold the proj bias into the
                # residual in place so the epilogue is a single add
                nc.vector.tensor_add(x2[:, u, :], x2[:, u, :], bproj_b[:])

            def _emit_ln2(u):
                ot = cin.tile([128, H], BF16, tag="ot")
                if u < 3:
                    nc.sync.dma_start(ot[:], oshard[0][128 * u : 128 * (u + 1), :])
                else:
                    nc.sync.dma_start(ot[:], oshard[1][:])
                xst = cin.tile([128, H], F32, tag="xst")
                nc.sync.dma_start(xst[:], xs[128 * u : 128 * (u + 1), :])
                nc.vector.tensor_add(x2[:, u, :], ot[:], xst[:])
                return (u, _ln_stage1(nc, lnpools, x2[:, u, :]))

            def _emit_fc(half, pf_alloc):
                q0 = 256 * half
                for g in range(8):
                    if half == 0 and g < 2:
                        wt = wfc_pre[g]
                    else:
                        wt = wstream.tile([128, 8, TS], BF16, tag="wst",
                                          name=f"wfc{half}_{g}")
                        nc.sync.dma_start(
                            wt[:],
                            wfc.rearrange("(h p) f -> p h f", p=128)[
                                :, :, TS * g : TS * (g + 1)
                            ],
                        )
                    for c4 in range(4):
                        fc = 4 * g + c4
                        pf = pf_alloc(f"pf{half}_{fc}")
                        for ht in range(8):
                            nc.tensor.matmul(
                                pf[:],
                                wt[:, ht, 128 * c4 : 128 * (c4 + 1)],
                                ln2T[:, ht, q0 : q0 + 256],
                                start=(ht == 0),
                                stop=(ht == 7),
                            )
                        nc.scalar.activation(
                            h1T[:, fc, q0 : q0 + 256], pf[:], AF.Gelu,
                            bias=bfc_sb[:, fc : fc + 1],
                        )

            def _emit_proj(half, accs):
                us = (0, 1) if half == 0 else (2, 3)
                for g in range(8):
                    wt = wstream.tile([128, 4, H], BF16, tag="wst",
                                      name=f"wpj{half}_{g}")
                    nc.sync.dma_start(
                        wt[:],
                        wproj.rearrange("(c p) f -> p c f", p=128)[
                            :, 4 * g : 4 * (g + 1), :
                        ],
                    )
                    for iu, u in enumerate(us):
                        for oc in range(2):
                            for f4 in range(4):
                                fc = 4 * g + f4
                                nc.tensor.matmul(
                                    accs[2 * iu + oc][:],
                                    h1T[:, fc, 128 * u : 128 * (u + 1)],
                                    wt[:, f4, TS * oc : TS * (oc + 1)],
                                    start=(g == 0 and f4 == 0),
                                    stop=(g == 7 and f4 == 3),
                                    skip_group_check=True,
                                )
                            if g == 7:
                                # drain each finished accumulator while the
                                # PE works on the remaining pairs
                                osl = slice(TS * oc, TS * (oc + 1))
                                ro = outp.tile([128, TS], F32, tag="ro",
                                               name=f"ro{u}_{oc}")
                                nc.vector.tensor_add(
                                    ro[:], accs[2 * iu + oc][:],
                                    x2[:, u, osl])
                                nc.sync.dma_start(
                                    out[128 * u : 128 * (u + 1), osl], ro[:])

            pend2 = [_emit_ln2(0)]
            pend2.append(_emit_ln2(1))
            _finish_ln2(*pend2.pop(0))
            pend2.append(_emit_ln2(2))
            _finish_ln2(*pend2.pop(0))
            _emit_fc(0, lambda nm: ps512.tile([128, 256], F32, tag="s",
                                              name=nm))
            accs0 = [ps512.tile([128, TS], F32, tag="s", name=f"acc0_{i}")
                     for i in range(4)]
            _emit_proj(0, accs0)
            pend2.append(_emit_ln2(3))
            _finish_ln2(*pend2.pop(0))
            _finish_ln2(*pend2.pop(0))
            _emit_fc(1, lambda nm: psT.tile([128, 256], F32, tag="pt",
                                            name=nm))
            accs1 = [psT.tile([128, TS], F32, tag="pt", name="acc1_0"),
                     psT.tile([128, TS], F32, tag="pt", name="acc1_1"),
                     psAV.tile([128, TS], F32, tag="pav", name="acc1_2"),
                     psAV.tile([128, TS], F32, tag="pav", name="acc1_3")]
            _emit_proj(1, accs1)


_CACHE = {}


def _get_compiled():
    if "nc" not in _CACHE:
        nc = bacc.Bacc("TRN2", target_bir_lowering=False, debug=False,
                       num_devices=NCORES)
        build(nc)
        nc.compile()
        _CACHE["nc"] = nc
    return _CACHE["nc"]


def make_in_maps(x, ln1_w, ln1_b, W_qkv, b_qkv, W_o, b_o, ln2_w, ln2_b, W_fc,
                 b_fc, W_proj, b_proj):
    x = np.ascontiguousarray(np.asarray(x, np.float32))
    ln1_w = np.asarray(ln1_w, np.float32)
    ln1_b = np.asarray(ln1_b, np.float32)
    ln2_w = np.asarray(ln2_w, np.float32)
    ln2_b = np.asarray(ln2_b, np.float32)
    W_qkv_raw = np.asarray(W_qkv, np.float32)
    W_fc_raw = np.asarray(W_fc, np.float32)
    # fold the layernorm affines into the downstream projections:
    # (c*w + b) @ W == c @ (diag(w) W) + b @ W
    W_qkv = ln1_w[:, None] * W_qkv_raw
    b_qkv = np.asarray(b_qkv, np.float32) + ln1_b @ W_qkv_raw
    bf = lambda a: np.ascontiguousarray(np.asarray(a).astype(NPBF16))
    shared = {
        "wfc": bf(ln2_w[:, None] * W_fc_raw),
        "bfc": np.ascontiguousarray(
            (np.asarray(b_fc, np.float32) + ln2_b @ W_fc_raw
             ).reshape(FF // 128, 128)),
        "wproj": bf(W_proj),
        "bproj": bf(np.asarray(b_proj, np.float32).reshape(1, H)),
    }
    in_maps = []
    for c in range(NCORES):
        b, r = c // TP, c % TP
        fsl = slice(FQ * r, FQ * (r + 1))
        m = dict(shared)
        m["x"] = bf(x[b])
        m["xs"] = np.ascontiguousarray(np.concatenate(
            [x[b][TS * u + 128 * r : TS * u + 128 * (r + 1)]
             for u in range(4)], axis=0) + np.asarray(b_o, np.float32)[None, :])
        m["wq"] = bf(W_qkv[:, fsl])
        m["wk"] = bf(W_qkv[:, H:][:, fsl])
        m["wv"] = bf(W_qkv[:, 2 * H :][:, fsl])
        m["bq"] = np.ascontiguousarray(b_qkv[fsl].reshape(2, 128))
        m["bk"] = np.ascontiguousarray(b_qkv[H:][fsl].reshape(2, 128))
        m["bv"] = bf(b_qkv[2 * H :][fsl].reshape(1, FQ))
        m["wo"] = bf(np.asarray(W_o, np.float32)[fsl, :])
        in_maps.append(m)
    return in_maps


def kernel(**inputs):
    nc = _get_compiled()
    in_maps = make_in_maps(**inputs)
    res = bass_utils.run_bass_kernel_spmd(
        nc, in_maps, core_ids=list(range(NCORES)), trace=False
    )
    out = np.empty((B, S, H), np.float32)
    for c in range(NCORES):
        b, r = c // TP, c % TP
        o = res.results[c]["out"]
        for u in range(4):
            out[b, TS * u + 128 * r : TS * u + 128 * (r + 1), :] = \
                o[128 * u : 128 * (u + 1)]
    return out
